# revision 20
# baseline (speedup 1.0000x reference)
"""BiGRU encoder kernel for 8 Trainium2 NeuronCores.

Strategy (v2 — mixed fp8/fp32r):
  - Same ragged reformulation as v1: masked GRUs over FIXED position ranges
    (fwd 0..7 ascending, bwd 14..7 descending); sort samples by window_len,
    deal round-robin to 8 cores; per core two batch tiles of 512; each GRU
    step runs on the suffix of samples long enough to need it.
  - Freeze semantics via the UPDATE-gate complement: h' = h + z~*(n - h)
    with z~ = sigmoid(-(pre_z)).  Pre-start samples get +2560 added to the
    raw pre_z, so z~ = sigmoid(-40) ~ 4e-18 and h' rounds back to h exactly
    even in bf16 (v1's h' = n + z*(h-n) would drift in bf16).
  - Tensor-engine mixed precision (sim rel-err 0.93% vs 2e-2 budget):
      * r-gate input+hidden and z/n-gate hidden matmuls: fp8e4m3 with
        MatmulPerfMode.DoubleRow (K=256 per instruction, 2x fp32r rate).
      * z/n-gate input matmuls and the MLP: fp32r (z-input and MLP are the
        accuracy-critical paths; n-input close behind).
    Weights are pre-scaled by 64 host-side (fp8 subnormal avoidance) and the
    1/64 folds into the activation `scale`.
  - fp8 DoubleRow has no 256-wide rate cliff, so fp8 matmuls run at exact
    suffix widths (64-granular); fp32r keeps the >=256 clamp; hidden-side
    matmuls use the PREVIOUS step's count (h==0 for just-started samples,
    and over-included samples are exact because start=True clears the PSUM
    bank and z~=0 freezes them).
  - h carried in bf16 (elementwise chain gets 2x DVE); a per-step fp8 copy
    of h (gpsimd) feeds the hidden-side DoubleRow matmuls; final h in fp32
    for the fp32r MLP.  Mask-add moved to gpsimd.
"""

import os
from contextlib import ExitStack

import numpy as np
import ml_dtypes

import concourse.bacc as bacc
import concourse.tile as tile
from concourse import mybir
from concourse.bass_utils import run_bass_kernel_spmd
from concourse.masks import make_identity

NCORES = 8
B, T, D, H = 8192, 15, 512, 512
G = 3 * H
SW = 64.0          # weight pre-scale
BIGM = 40.0 * SW   # mask value on the 64-scaled pre-activation
S = 512
F32 = mybir.dt.float32
F32R = mybir.dt.float32r
BF16 = mybir.dt.bfloat16
F8 = mybir.dt.float8e4
DR = mybir.MatmulPerfMode.DoubleRow
E4NP = ml_dtypes.float8_e4m3

ACT = mybir.ActivationFunctionType
ALU = mybir.AluOpType

_PROGRAM_CACHE = {}
LAST_RESULT = None


def _build_program(sched):
    """sched: per tile, (f_steps, b_steps); step = (wr, wz, wh, masked)."""
    ntiles = len(sched)
    Bc = S * ntiles
    nc = bacc.Bacc("TRN2", target_bir_lowering=False, debug=False,
                   num_devices=NCORES)

    xT_d = nc.dram_tensor("xT", [T, D, Bc], F32R, kind="ExternalInput")
    x8T_d = nc.dram_tensor("xT8", [T, D, Bc], F8, kind="ExternalInput")
    wzn_f_d = nc.dram_tensor("wznf", [D, 2 * H], F32R, kind="ExternalInput")
    wzn_b_d = nc.dram_tensor("wznb", [D, 2 * H], F32R, kind="ExternalInput")
    w8i_f_d = nc.dram_tensor("w8if", [D, H], F8, kind="ExternalInput")
    w8i_b_d = nc.dram_tensor("w8ib", [D, H], F8, kind="ExternalInput")
    w8h_f_d = nc.dram_tensor("w8hf", [H, G], F8, kind="ExternalInput")
    w8h_b_d = nc.dram_tensor("w8hb", [H, G], F8, kind="ExternalInput")
    w1_d = nc.dram_tensor("w1", [2 * H, H], F32R, kind="ExternalInput")
    w2_d = nc.dram_tensor("w2", [H, H], F32R, kind="ExternalInput")
    bias_d = nc.dram_tensor("bias", [40, 128], F32, kind="ExternalInput")
    mf_d = nc.dram_tensor("maskzf", [8, Bc], F32, kind="ExternalInput")
    mb_d = nc.dram_tensor("maskzb", [8, Bc], F32, kind="ExternalInput")
    y_d = nc.dram_tensor("y", [Bc, H], F32, kind="ExternalOutput")
    dbg = bool(os.environ.get("GRU_DBG"))
    if dbg:
        hf_d = nc.dram_tensor("hfdbg", [ntiles, 128, 4, S], F32,
                              kind="ExternalOutput")
        hb_d = nc.dram_tensor("hbdbg", [ntiles, 128, 4, S], F32,
                              kind="ExternalOutput")

    with tile.TileContext(nc) as tc, ExitStack() as ctx:
        const = ctx.enter_context(tc.tile_pool(name="const", bufs=1))
        wpool = ctx.enter_context(tc.tile_pool(name="w", bufs=2))
        w8pool = ctx.enter_context(tc.tile_pool(name="w8", bufs=4))
        xpool = ctx.enter_context(tc.tile_pool(name="x", bufs=3))
        x8pool = ctx.enter_context(tc.tile_pool(name="x8", bufs=3))
        hpool = ctx.enter_context(tc.tile_pool(name="h", bufs=2))
        h8pool = ctx.enter_context(tc.tile_pool(name="h8", bufs=2))
        hfin = ctx.enter_context(tc.tile_pool(name="hfin", bufs=4))
        gpool = ctx.enter_context(tc.tile_pool(name="g", bufs=6))
        mpool = ctx.enter_context(tc.tile_pool(name="m", bufs=2))
        opool = ctx.enter_context(tc.tile_pool(name="o", bufs=4))
        rzps = ctx.enter_context(tc.tile_pool(name="rz", bufs=4, space="PSUM"))
        xpps = ctx.enter_context(tc.tile_pool(name="xp", bufs=2, space="PSUM"))
        ghps = ctx.enter_context(tc.tile_pool(name="gh", bufs=2, space="PSUM"))

        def load_w(dram, kchunks, cols, name, dt=F32R, pool=None, tag=None,
                   spread=False):
            t_ = (pool or wpool).tile([128, kchunks, cols], dt,
                                      tag=tag or ("w" if pool is None else "const"),
                                      name=name)
            src = dram.rearrange("(c k) g -> k c g", k=128)
            engs = (nc.sync, nc.scalar, nc.gpsimd)
            for c in range(kchunks):
                if spread:
                    # split each chunk's columns across all queues: the DMA
                    # engines run in parallel, 4x the load bandwidth
                    hc = cols // 2
                    engs[(2 * c) % 3].dma_start(t_[:, c, :hc], src[:, c, :hc])
                    engs[(2 * c + 1) % 3].dma_start(t_[:, c, hc:], src[:, c, hc:])
                else:
                    engs[c % 2].dma_start(t_[:, c, :], src[:, c, :])
            return t_

        # fp8 weights first: tiny DMAs, lets the r-gate DR matmuls start early
        w8i_f = load_w(w8i_f_d, 4, H, "w8if", dt=F8, pool=w8pool, tag="w8")
        w8h_f = load_w(w8h_f_d, 4, G, "w8hf", dt=F8, pool=w8pool, tag="w8")
        wzn_f = load_w(wzn_f_d, 4, 2 * H, "wznf", spread=True)
        w8i_b = load_w(w8i_b_d, 4, H, "w8ib", dt=F8, pool=w8pool, tag="w8")
        w8h_b = load_w(w8h_b_d, 4, G, "w8hb", dt=F8, pool=w8pool, tag="w8")
        wzn_b = load_w(wzn_b_d, 4, 2 * H, "wznb", spread=True)
        w2 = load_w(w2_d, 4, H, "w2", pool=const)
        bt = const.tile([128, 40], F32)
        nc.gpsimd.dma_start(bt[:], bias_d.rearrange("n p -> p n"))
        ident = const.tile([128, 128], F32)
        make_identity(nc, ident[:])

        def emit_dir(s0, steps, wzn, w8i, w8h, mask_d, bb, pos_fn):
            """One GRU direction over one batch tile; returns final h tile."""
            nsteps = len(steps)
            h_prev = None
            h8_prev = None
            for j, (wr, wz, wh, wmask) in enumerate(steps):
                first = j == 0
                last = j == nsteps - 1
                masked = wmask > 0
                p_abs = pos_fn(j)
                so = S - wr    # chain/suffix offset
                soz = S - wz
                soh = S - wh if wh else S
                x8 = x8pool.tile([128, 4, S], F8, tag="x8", name="x8")
                nc.sync.dma_start(
                    x8[:, :, so:],
                    x8T_d[p_abs].rearrange("(c k) s -> k c s", k=128)[:, :, s0 + so:s0 + S],
                )
                xt = xpool.tile([128, 4, S], F32R, tag="x", name="xt")
                nc.sync.dma_start(
                    xt[:, :, soz:],
                    xT_d[p_abs].rearrange("(c k) s -> k c s", k=128)[:, :, s0 + soz:s0 + S],
                )
                mt = None
                if masked:
                    mt = mpool.tile([128, S], F32, tag="m", name="mt")
                    nc.gpsimd.dma_start(
                        mt[:, :wmask],
                        mask_d[8 - nsteps + j,
                               s0 + so:s0 + so + wmask].partition_broadcast(128),
                    )
                h_next = (hfin if last else hpool).tile(
                    [128, 4, S], F32R if last else BF16,
                    tag="hf" if last else "h", name="h")
                h8_next = None
                if not last:
                    h8_next = h8pool.tile([128, 4, S], F8, tag="h8", name="h8")
                    nwr = steps[j + 1][0]
                    if S - nwr < so:  # next step includes more samples
                        meng = nc.vector if os.environ.get("GRU_MS") == "v" else nc.gpsimd
                        meng.memset(h_next[:, :, S - nwr:so], 0.0)

                for i in range(4):
                    r_ps = rzps.tile([128, wr], F32, tag="rz", name=f"rps{i}")
                    z_ps = rzps.tile([128, wz], F32, tag="rz", name=f"zps{i}")
                    xpn = xpps.tile([128, wz], F32, tag="xp", name=f"xpn{i}")
                    # r-gate input: fp8 DoubleRow over 2 k-pairs
                    for p in range(2):
                        nc.tensor.matmul(
                            r_ps[:], w8i[:, 2 * p:2 * p + 2, i * 128:(i + 1) * 128],
                            x8[:, 2 * p:2 * p + 2, so:],
                            start=p == 0, stop=first and p == 1, perf_mode=DR)
                    # z/n input: fp32r
                    for k in range(4):
                        nc.tensor.matmul(z_ps[:], wzn[:, k, i * 128:(i + 1) * 128],
                                         xt[:, k, soz:], start=k == 0,
                                         stop=first and k == 3)
                        nc.tensor.matmul(xpn[:],
                                         wzn[:, k, H + i * 128:H + (i + 1) * 128],
                                         xt[:, k, soz:], start=k == 0, stop=k == 3)
                    ghn = None
                    if not first:
                        ghn = ghps.tile([128, wh], F32, tag="gh", name=f"ghn{i}")
                        for p in range(2):
                            hs = h8_prev[:, 2 * p:2 * p + 2, soh:]
                            nc.tensor.matmul(
                                r_ps[:, wr - wh:],
                                w8h[:, 2 * p:2 * p + 2, i * 128:(i + 1) * 128],
                                hs, start=False, stop=p == 1, perf_mode=DR)
                            nc.tensor.matmul(
                                z_ps[:, wz - wh:],
                                w8h[:, 2 * p:2 * p + 2, H + i * 128:H + (i + 1) * 128],
                                hs, start=False, stop=p == 1, perf_mode=DR)
                            nc.tensor.matmul(
                                ghn[:],
                                w8h[:, 2 * p:2 * p + 2, 2 * H + i * 128:2 * H + (i + 1) * 128],
                                hs, start=p == 0, stop=p == 1, perf_mode=DR)

                    r = gpool.tile([128, wr], BF16, tag="g", name="r")
                    nc.scalar.activation(r[:], r_ps[:], ACT.Sigmoid,
                                         bias=bt[:, bb + i:bb + i + 1],
                                         scale=1.0 / SW)
                    if masked:
                        # freeze the over-included prefix: in-place +2560 on
                        # the 64-scaled z pre-activation (-> z~ = sigmoid(-40))
                        zp = z_ps[:, wz - wr:wz - wr + wmask]
                        if os.environ.get("GRU_MA") == "t":
                            zin = gpool.tile([128, wmask], F32, tag="g", name="zin")
                            nc.vector.tensor_add(zin[:], zp, mt[:, :wmask])
                            nc.vector.tensor_copy(zp, zin[:])
                        else:
                            nc.vector.tensor_add(zp, zp, mt[:, :wmask])
                    z = gpool.tile([128, wr], BF16, tag="g", name="z")
                    nc.scalar.activation(z[:], z_ps[:, wz - wr:], ACT.Sigmoid,
                                         bias=bt[:, bb + 4 + i:bb + 5 + i],
                                         scale=-1.0 / SW)
                    tt = gpool.tile([128, wr], BF16, tag="g", name="tt")
                    if first:
                        nc.vector.tensor_scalar_mul(tt[:], r[:],
                                                    bt[:, bb + 8 + i:bb + 9 + i])
                    else:
                        if wh < wr:
                            # just-started samples: h_prev == 0, so the hidden
                            # n-term is exactly the bhh_n bias
                            nc.vector.tensor_scalar_mul(
                                tt[:, :wr - wh], r[:, :wr - wh],
                                bt[:, bb + 8 + i:bb + 9 + i])
                        nc.vector.scalar_tensor_tensor(
                            tt[:, wr - wh:], ghn[:],
                            bt[:, bb + 8 + i:bb + 9 + i], r[:, wr - wh:],
                            op0=ALU.add, op1=ALU.mult)
                    ss = gpool.tile([128, wr], BF16, tag="g", name="ss")
                    nc.vector.tensor_add(ss[:], tt[:], xpn[:, wz - wr:])
                    n = gpool.tile([128, wr], BF16, tag="g", name="n")
                    nc.scalar.activation(n[:], ss[:], ACT.Tanh,
                                         bias=bt[:, bb + 12 + i:bb + 13 + i],
                                         scale=1.0 / SW)
                    ho = h_next[:, i, so:]
                    if first:
                        nc.vector.tensor_mul(ho, z[:], n[:])
                    else:
                        dd = gpool.tile([128, wr], BF16, tag="g", name="dd")
                        nc.vector.tensor_sub(dd[:], n[:], h_prev[:, i, so:])
                        e = gpool.tile([128, wr], BF16, tag="g", name="e")
                        nc.vector.tensor_mul(e[:], z[:], dd[:])
                        nc.vector.tensor_add(ho, h_prev[:, i, so:], e[:])
                    if not last:
                        # fp8 copy for next step's hidden matmuls; scalar engine
                        # casts ~5x faster than gpsimd and is off-critical here
                        nc.scalar.activation(h8_next[:, i, so:], ho, ACT.Copy)
                h_prev = h_next
                h8_prev = h8_next
            return h_prev

        hfs = []
        for t in range(ntiles):
            nf = len(sched[t][0])
            hfs.append(emit_dir(t * S, sched[t][0], wzn_f, w8i_f, w8h_f, mf_d,
                                0, lambda j, nf=nf: 8 - nf + j))
            if dbg:
                nc.sync.dma_start(hf_d[t], hfs[t][:].bitcast(F32))
        hbs = []
        for t in range(ntiles):
            nb = len(sched[t][1])
            hbs.append(emit_dir(t * S, sched[t][1], wzn_b, w8i_b, w8h_b, mb_d,
                                16, lambda j, nb=nb: 6 + nb - j))
            if dbg:
                nc.sync.dma_start(hb_d[t], hbs[t][:].bitcast(F32))
        w1 = load_w(w1_d, 8, H, "w1")

        def emit_mlp(t, hf, hb):
            hid = []
            for i in range(4):
                ps = xpps.tile([128, S], F32, tag="xp", name="mps")
                for k in range(8):
                    src = hf if k < 4 else hb
                    nc.tensor.matmul(ps[:], w1[:, k, i * 128:(i + 1) * 128],
                                     src[:, k % 4, :], start=k == 0, stop=k == 7)
                h32 = gpool.tile([128, S], F32, tag="g", name="h32")
                nc.scalar.activation(h32[:], ps[:], ACT.Relu,
                                     bias=bt[:, 32 + i:33 + i])
                hr = gpool.tile([128, S], F32R, tag="g", name="hr")
                nc.vector.tensor_copy(hr[:], h32[:])
                hid.append(hr)
            onats = []
            for gidx in range(S // 128):
                onat = opool.tile([128, H], F32, tag="o", name=f"onat{gidx}")
                onats.append(onat)
            for i in range(4):
                ps = xpps.tile([128, S], F32, tag="xp", name="ops")
                for k in range(4):
                    nc.tensor.matmul(ps[:], w2[:, k, i * 128:(i + 1) * 128],
                                     hid[k][:], start=k == 0, stop=k == 3)
                o32 = gpool.tile([128, S], F32, tag="g", name="o32")
                nc.vector.tensor_scalar_add(o32[:], ps[:], bt[:, 36 + i:37 + i])
                for gidx in range(S // 128):
                    tp = ghps.tile([128, 128], F32, tag="gh", name="tp")
                    nc.tensor.transpose(tp[:], o32[:, gidx * 128:(gidx + 1) * 128],
                                        ident[:])
                    nc.vector.tensor_copy(onats[gidx][:, i * 128:(i + 1) * 128],
                                          tp[:])
            for gidx in range(S // 128):
                r0 = t * S + gidx * 128
                nc.sync.dma_start(y_d[r0:r0 + 128, :], onats[gidx][:])

        for t in range(ntiles):
            emit_mlp(t, hfs[t], hbs[t])

    nc.compile()
    return nc


def kernel(padded_window, window_len, Wih_f, Whh_f, bih_f, bhh_f,
           Wih_b, Whh_b, bih_b, bhh_b, W1, b1, W2, b2):
    wl = np.asarray(window_len)
    lf = (wl - 1) // 2 + 1
    lb = wl // 2 + 1
    order = np.argsort(wl, kind="stable")

    Bc = B // NCORES
    ntiles = Bc // S
    lf_pc = lf[order].reshape(-1, NCORES)
    lb_pc = lb[order].reshape(-1, NCORES)

    def r64(v):
        return int(min(S, -(-int(v) // 64) * 64))

    mode = os.environ.get("GRU_SCHED", "exact")

    def dir_steps(lens_pc, t):
        seg = lens_pc[t * S:(t + 1) * S]  # [S, NCORES]
        n = int(seg.max())
        steps = []
        prev_cmax = 0
        prev_wr = 0
        for j in range(n):
            need = n - j
            cnt = (seg >= need).sum(axis=0)
            cmax, cmin = int(cnt.max()), int(cnt.min())
            if mode == "v1ish":
                wr = wz = int(min(S, max(256, r64(cmax))))
                wh = prev_wr if j > 0 else 0
            elif mode == "a":  # exact wr, wz==wr, wide wh
                wr = wz = r64(cmax)
                wh = prev_wr if j > 0 else 0
            elif mode == "b":  # clamped wr, narrow wh
                wr = wz = int(min(S, max(256, r64(cmax))))
                wh = r64(prev_cmax) if j > 0 else 0
            else:
                wr = r64(cmax)
                wz = wr if wr <= 128 else max(256, wr)
                if mode == "nowh":
                    wh = prev_wr if j > 0 else 0
                else:
                    wh = r64(prev_cmax) if j > 0 else 0
            wmask = wr - cmin  # width of the over-included (to-freeze) prefix
            steps.append((wr, wz, wh, wmask))
            prev_cmax = cmax
            prev_wr = wr
        return tuple(steps)

    sched = tuple((dir_steps(lf_pc, t), dir_steps(lb_pc, t))
                  for t in range(ntiles))

    cache_key = (sched, bool(os.environ.get("GRU_DBG")))
    if cache_key not in _PROGRAM_CACHE:
        _PROGRAM_CACHE[cache_key] = _build_program(sched)
    nc = _PROGRAM_CACHE[cache_key]

    f32 = np.float32
    WihfT, WhhfT = Wih_f.T.astype(f32), Whh_f.T.astype(f32)
    WihbT, WhhbT = Wih_b.T.astype(f32), Whh_b.T.astype(f32)
    wzn_f = np.ascontiguousarray(WihfT[:, H:] * SW)
    wzn_b = np.ascontiguousarray(WihbT[:, H:] * SW)
    w8i_f = (WihfT[:, :H] * SW).astype(E4NP)
    w8i_b = (WihbT[:, :H] * SW).astype(E4NP)
    w8h_f = (WhhfT * SW).astype(E4NP)
    w8h_b = (WhhbT * SW).astype(E4NP)
    w1 = np.ascontiguousarray(W1.T, dtype=f32)
    w2 = np.ascontiguousarray(W2.T, dtype=f32)

    def chunks(v):  # [512] -> [4, 128]
        return np.asarray(v, f32).reshape(4, 128)

    bias = np.concatenate([
        chunks((bih_f + bhh_f)[:H]), chunks(-(bih_f + bhh_f)[H:2 * H]),
        chunks(SW * bhh_f[2 * H:]), chunks(bih_f[2 * H:]),
        chunks((bih_b + bhh_b)[:H]), chunks(-(bih_b + bhh_b)[H:2 * H]),
        chunks(SW * bhh_b[2 * H:]), chunks(bih_b[2 * H:]),
        chunks(b1), chunks(b2),
    ], 0)  # [40, 128]

    pw = np.asarray(padded_window, f32)
    in_maps = []
    p8 = np.arange(8)
    for c in range(NCORES):
        idx = order[c::NCORES]
        xT = np.ascontiguousarray(pw[idx].transpose(1, 2, 0))  # [15, 512, Bc]
        mzf = (BIGM * (p8[:, None] < (8 - lf[idx])[None, :])).astype(f32)
        mzb = (BIGM * (p8[:, None] < (8 - lb[idx])[None, :])).astype(f32)
        in_maps.append({
            "xT": xT, "xT8": xT.astype(E4NP),
            "wznf": wzn_f, "wznb": wzn_b,
            "w8if": w8i_f, "w8ib": w8i_b, "w8hf": w8h_f, "w8hb": w8h_b,
            "w1": w1, "w2": w2,
            "bias": bias, "maskzf": mzf, "maskzb": mzb,
        })

    trace = bool(os.environ.get("GRU_TRACE"))
    kw = {}
    if os.environ.get("GRU_TMPDIR"):
        kw["tmpdir"] = os.environ["GRU_TMPDIR"]
    res = run_bass_kernel_spmd(nc, in_maps, core_ids=list(range(NCORES)),
                               trace=trace, **kw)
    global LAST_RESULT
    LAST_RESULT = res
    out = np.empty((B, H), f32)
    for c in range(NCORES):
        out[order[c::NCORES]] = res.results[c]["y"]
    return out


# revision 21
# speedup vs baseline: 1.0105x; 1.0105x over previous
"""BiGRU encoder kernel for 8 Trainium2 NeuronCores.

Strategy (v2 — mixed fp8/fp32r):
  - Same ragged reformulation as v1: masked GRUs over FIXED position ranges
    (fwd 0..7 ascending, bwd 14..7 descending); sort samples by window_len,
    deal round-robin to 8 cores; per core two batch tiles of 512; each GRU
    step runs on the suffix of samples long enough to need it.
  - Freeze semantics via the UPDATE-gate complement: h' = h + z~*(n - h)
    with z~ = sigmoid(-(pre_z)).  Pre-start samples get +2560 added to the
    raw pre_z, so z~ = sigmoid(-40) ~ 4e-18 and h' rounds back to h exactly
    even in bf16 (v1's h' = n + z*(h-n) would drift in bf16).
  - Tensor-engine mixed precision (sim rel-err 0.93% vs 2e-2 budget):
      * r-gate input+hidden and z/n-gate hidden matmuls: fp8e4m3 with
        MatmulPerfMode.DoubleRow (K=256 per instruction, 2x fp32r rate).
      * z/n-gate input matmuls and the MLP: fp32r (z-input and MLP are the
        accuracy-critical paths; n-input close behind).
    Weights are pre-scaled by 64 host-side (fp8 subnormal avoidance) and the
    1/64 folds into the activation `scale`.
  - fp8 DoubleRow has no 256-wide rate cliff, so fp8 matmuls run at exact
    suffix widths (64-granular); fp32r keeps the >=256 clamp; hidden-side
    matmuls use the PREVIOUS step's count (h==0 for just-started samples,
    and over-included samples are exact because start=True clears the PSUM
    bank and z~=0 freezes them).
  - h carried in bf16 (elementwise chain gets 2x DVE); a per-step fp8 copy
    of h (gpsimd) feeds the hidden-side DoubleRow matmuls; final h in fp32
    for the fp32r MLP.  Mask-add moved to gpsimd.
"""

import os
from contextlib import ExitStack

import numpy as np
import ml_dtypes

import concourse.bacc as bacc
import concourse.tile as tile
from concourse import mybir
from concourse.bass_utils import run_bass_kernel_spmd
from concourse.masks import make_identity

NCORES = 8
B, T, D, H = 8192, 15, 512, 512
G = 3 * H
SW = 64.0          # weight pre-scale
BIGM = 40.0 * SW   # mask value on the 64-scaled pre-activation
S = 512
F32 = mybir.dt.float32
F32R = mybir.dt.float32r
BF16 = mybir.dt.bfloat16
F8 = mybir.dt.float8e4
DR = mybir.MatmulPerfMode.DoubleRow
E4NP = ml_dtypes.float8_e4m3

ACT = mybir.ActivationFunctionType
ALU = mybir.AluOpType

_PROGRAM_CACHE = {}
LAST_RESULT = None


def _build_program(sched):
    """sched: per tile, (f_steps, b_steps); step = (wr, wz, wh, masked)."""
    ntiles = len(sched)
    Bc = S * ntiles
    nc = bacc.Bacc("TRN2", target_bir_lowering=False, debug=False,
                   num_devices=NCORES)

    xT_d = nc.dram_tensor("xT", [T, D, Bc], F32R, kind="ExternalInput")
    x8T_d = nc.dram_tensor("xT8", [T, D, Bc], F8, kind="ExternalInput")
    wzn_f_d = nc.dram_tensor("wznf", [D, 2 * H], F32R, kind="ExternalInput")
    wzn_b_d = nc.dram_tensor("wznb", [D, 2 * H], F32R, kind="ExternalInput")
    w8i_f_d = nc.dram_tensor("w8if", [D, H], F8, kind="ExternalInput")
    w8i_b_d = nc.dram_tensor("w8ib", [D, H], F8, kind="ExternalInput")
    w8h_f_d = nc.dram_tensor("w8hf", [H, G], F8, kind="ExternalInput")
    w8h_b_d = nc.dram_tensor("w8hb", [H, G], F8, kind="ExternalInput")
    w1_d = nc.dram_tensor("w1", [2 * H, H], F32R, kind="ExternalInput")
    w2_d = nc.dram_tensor("w2", [H, H], F32R, kind="ExternalInput")
    bias_d = nc.dram_tensor("bias", [40, 128], F32, kind="ExternalInput")
    mf_d = nc.dram_tensor("maskzf", [8, Bc], F32, kind="ExternalInput")
    mb_d = nc.dram_tensor("maskzb", [8, Bc], F32, kind="ExternalInput")
    y_d = nc.dram_tensor("y", [Bc, H], F32, kind="ExternalOutput")
    dbg = bool(os.environ.get("GRU_DBG"))
    if dbg:
        hf_d = nc.dram_tensor("hfdbg", [ntiles, 128, 4, S], F32,
                              kind="ExternalOutput")
        hb_d = nc.dram_tensor("hbdbg", [ntiles, 128, 4, S], F32,
                              kind="ExternalOutput")

    with tile.TileContext(nc) as tc, ExitStack() as ctx:
        const = ctx.enter_context(tc.tile_pool(name="const", bufs=1))
        wpool = ctx.enter_context(tc.tile_pool(name="w", bufs=2))
        w8pool = ctx.enter_context(tc.tile_pool(name="w8", bufs=4))
        xpool = ctx.enter_context(tc.tile_pool(name="x", bufs=3))
        x8pool = ctx.enter_context(tc.tile_pool(name="x8", bufs=3))
        hpool = ctx.enter_context(tc.tile_pool(name="h", bufs=2))
        h8pool = ctx.enter_context(tc.tile_pool(name="h8", bufs=2))
        hfin = ctx.enter_context(tc.tile_pool(name="hfin", bufs=4))
        gpool = ctx.enter_context(tc.tile_pool(name="g", bufs=6))
        mpool = ctx.enter_context(tc.tile_pool(name="m", bufs=2))
        opool = ctx.enter_context(tc.tile_pool(name="o", bufs=4))
        rzps = ctx.enter_context(tc.tile_pool(name="rz", bufs=4, space="PSUM"))
        xpps = ctx.enter_context(tc.tile_pool(name="xp", bufs=2, space="PSUM"))
        ghps = ctx.enter_context(tc.tile_pool(name="gh", bufs=2, space="PSUM"))

        def load_w(dram, kchunks, cols, name, dt=F32R, pool=None, tag=None,
                   spread=False):
            t_ = (pool or wpool).tile([128, kchunks, cols], dt,
                                      tag=tag or ("w" if pool is None else "const"),
                                      name=name)
            src = dram.rearrange("(c k) g -> k c g", k=128)
            engs = (nc.sync, nc.gpsimd)
            for c in range(kchunks):
                if spread:
                    # split each chunk's columns across all queues: the DMA
                    # engines run in parallel, 4x the load bandwidth
                    hc = cols // 2
                    engs[0].dma_start(t_[:, c, :hc], src[:, c, :hc])
                    engs[1].dma_start(t_[:, c, hc:], src[:, c, hc:])
                else:
                    engs[c % 2].dma_start(t_[:, c, :], src[:, c, :])
            return t_

        # fp8 weights first: tiny DMAs, lets the r-gate DR matmuls start early
        w8i_f = load_w(w8i_f_d, 4, H, "w8if", dt=F8, pool=w8pool, tag="w8")
        w8h_f = load_w(w8h_f_d, 4, G, "w8hf", dt=F8, pool=w8pool, tag="w8")
        wzn_f = load_w(wzn_f_d, 4, 2 * H, "wznf", spread=True)
        w8i_b = load_w(w8i_b_d, 4, H, "w8ib", dt=F8, pool=w8pool, tag="w8")
        w8h_b = load_w(w8h_b_d, 4, G, "w8hb", dt=F8, pool=w8pool, tag="w8")
        wzn_b = load_w(wzn_b_d, 4, 2 * H, "wznb", spread=True)
        w2 = load_w(w2_d, 4, H, "w2", pool=const)
        bt = const.tile([128, 40], F32)
        nc.gpsimd.dma_start(bt[:], bias_d.rearrange("n p -> p n"))
        ident = const.tile([128, 128], F32)
        make_identity(nc, ident[:])

        def emit_dir(s0, steps, wzn, w8i, w8h, mask_d, bb, pos_fn):
            """One GRU direction over one batch tile; returns final h tile."""
            nsteps = len(steps)
            h_prev = None
            h8_prev = None
            for j, (wr, wz, wh, wmask) in enumerate(steps):
                first = j == 0
                last = j == nsteps - 1
                masked = wmask > 0
                p_abs = pos_fn(j)
                so = S - wr    # chain/suffix offset
                soz = S - wz
                soh = S - wh if wh else S
                x8 = x8pool.tile([128, 4, S], F8, tag="x8", name="x8")
                nc.sync.dma_start(
                    x8[:, :, so:],
                    x8T_d[p_abs].rearrange("(c k) s -> k c s", k=128)[:, :, s0 + so:s0 + S],
                )
                xt = xpool.tile([128, 4, S], F32R, tag="x", name="xt")
                nc.sync.dma_start(
                    xt[:, :, soz:],
                    xT_d[p_abs].rearrange("(c k) s -> k c s", k=128)[:, :, s0 + soz:s0 + S],
                )
                mt = None
                if masked:
                    mt = mpool.tile([128, S], F32, tag="m", name="mt")
                    nc.gpsimd.dma_start(
                        mt[:, :wmask],
                        mask_d[8 - nsteps + j,
                               s0 + so:s0 + so + wmask].partition_broadcast(128),
                    )
                h_next = (hfin if last else hpool).tile(
                    [128, 4, S], F32R if last else BF16,
                    tag="hf" if last else "h", name="h")
                h8_next = None
                if not last:
                    h8_next = h8pool.tile([128, 4, S], F8, tag="h8", name="h8")
                    nwr = steps[j + 1][0]
                    if S - nwr < so:  # next step includes more samples
                        meng = nc.vector if os.environ.get("GRU_MS") == "v" else nc.gpsimd
                        meng.memset(h_next[:, :, S - nwr:so], 0.0)

                for i in range(4):
                    r_ps = rzps.tile([128, wr], F32, tag="rz", name=f"rps{i}")
                    z_ps = rzps.tile([128, wz], F32, tag="rz", name=f"zps{i}")
                    xpn = xpps.tile([128, wz], F32, tag="xp", name=f"xpn{i}")
                    # r-gate input: fp8 DoubleRow over 2 k-pairs
                    for p in range(2):
                        nc.tensor.matmul(
                            r_ps[:], w8i[:, 2 * p:2 * p + 2, i * 128:(i + 1) * 128],
                            x8[:, 2 * p:2 * p + 2, so:],
                            start=p == 0, stop=first and p == 1, perf_mode=DR)
                    # z/n input: fp32r
                    for k in range(4):
                        nc.tensor.matmul(z_ps[:], wzn[:, k, i * 128:(i + 1) * 128],
                                         xt[:, k, soz:], start=k == 0,
                                         stop=first and k == 3)
                        nc.tensor.matmul(xpn[:],
                                         wzn[:, k, H + i * 128:H + (i + 1) * 128],
                                         xt[:, k, soz:], start=k == 0, stop=k == 3)
                    ghn = None
                    if not first:
                        ghn = ghps.tile([128, wh], F32, tag="gh", name=f"ghn{i}")
                        for p in range(2):
                            hs = h8_prev[:, 2 * p:2 * p + 2, soh:]
                            nc.tensor.matmul(
                                r_ps[:, wr - wh:],
                                w8h[:, 2 * p:2 * p + 2, i * 128:(i + 1) * 128],
                                hs, start=False, stop=p == 1, perf_mode=DR)
                            nc.tensor.matmul(
                                z_ps[:, wz - wh:],
                                w8h[:, 2 * p:2 * p + 2, H + i * 128:H + (i + 1) * 128],
                                hs, start=False, stop=p == 1, perf_mode=DR)
                            nc.tensor.matmul(
                                ghn[:],
                                w8h[:, 2 * p:2 * p + 2, 2 * H + i * 128:2 * H + (i + 1) * 128],
                                hs, start=p == 0, stop=p == 1, perf_mode=DR)

                    r = gpool.tile([128, wr], BF16, tag="g", name="r")
                    nc.scalar.activation(r[:], r_ps[:], ACT.Sigmoid,
                                         bias=bt[:, bb + i:bb + i + 1],
                                         scale=1.0 / SW)
                    if masked:
                        # freeze the over-included prefix: in-place +2560 on
                        # the 64-scaled z pre-activation (-> z~ = sigmoid(-40))
                        zp = z_ps[:, wz - wr:wz - wr + wmask]
                        if os.environ.get("GRU_MA") == "t":
                            zin = gpool.tile([128, wmask], F32, tag="g", name="zin")
                            nc.vector.tensor_add(zin[:], zp, mt[:, :wmask])
                            nc.vector.tensor_copy(zp, zin[:])
                        else:
                            nc.vector.tensor_add(zp, zp, mt[:, :wmask])
                    z = gpool.tile([128, wr], BF16, tag="g", name="z")
                    nc.scalar.activation(z[:], z_ps[:, wz - wr:], ACT.Sigmoid,
                                         bias=bt[:, bb + 4 + i:bb + 5 + i],
                                         scale=-1.0 / SW)
                    tt = gpool.tile([128, wr], BF16, tag="g", name="tt")
                    if first:
                        nc.vector.tensor_scalar_mul(tt[:], r[:],
                                                    bt[:, bb + 8 + i:bb + 9 + i])
                    else:
                        if wh < wr:
                            # just-started samples: h_prev == 0, so the hidden
                            # n-term is exactly the bhh_n bias
                            nc.vector.tensor_scalar_mul(
                                tt[:, :wr - wh], r[:, :wr - wh],
                                bt[:, bb + 8 + i:bb + 9 + i])
                        nc.vector.scalar_tensor_tensor(
                            tt[:, wr - wh:], ghn[:],
                            bt[:, bb + 8 + i:bb + 9 + i], r[:, wr - wh:],
                            op0=ALU.add, op1=ALU.mult)
                    ss = gpool.tile([128, wr], BF16, tag="g", name="ss")
                    nc.vector.tensor_add(ss[:], tt[:], xpn[:, wz - wr:])
                    n = gpool.tile([128, wr], BF16, tag="g", name="n")
                    nc.scalar.activation(n[:], ss[:], ACT.Tanh,
                                         bias=bt[:, bb + 12 + i:bb + 13 + i],
                                         scale=1.0 / SW)
                    ho = h_next[:, i, so:]
                    if first:
                        nc.vector.tensor_mul(ho, z[:], n[:])
                    else:
                        dd = gpool.tile([128, wr], BF16, tag="g", name="dd")
                        nc.vector.tensor_sub(dd[:], n[:], h_prev[:, i, so:])
                        e = gpool.tile([128, wr], BF16, tag="g", name="e")
                        nc.vector.tensor_mul(e[:], z[:], dd[:])
                        nc.vector.tensor_add(ho, h_prev[:, i, so:], e[:])
                    if not last:
                        # fp8 copy for next step's hidden matmuls; scalar engine
                        # casts ~5x faster than gpsimd and is off-critical here
                        nc.scalar.activation(h8_next[:, i, so:], ho, ACT.Copy)
                h_prev = h_next
                h8_prev = h8_next
            return h_prev

        hfs = []
        for t in range(ntiles):
            nf = len(sched[t][0])
            hfs.append(emit_dir(t * S, sched[t][0], wzn_f, w8i_f, w8h_f, mf_d,
                                0, lambda j, nf=nf: 8 - nf + j))
            if dbg:
                nc.sync.dma_start(hf_d[t], hfs[t][:].bitcast(F32))
        hbs = []
        for t in range(ntiles):
            nb = len(sched[t][1])
            hbs.append(emit_dir(t * S, sched[t][1], wzn_b, w8i_b, w8h_b, mb_d,
                                16, lambda j, nb=nb: 6 + nb - j))
            if dbg:
                nc.sync.dma_start(hb_d[t], hbs[t][:].bitcast(F32))
        w1 = load_w(w1_d, 8, H, "w1")

        def emit_mlp(t, hf, hb):
            hid = []
            for i in range(4):
                ps = xpps.tile([128, S], F32, tag="xp", name="mps")
                for k in range(8):
                    src = hf if k < 4 else hb
                    nc.tensor.matmul(ps[:], w1[:, k, i * 128:(i + 1) * 128],
                                     src[:, k % 4, :], start=k == 0, stop=k == 7)
                h32 = gpool.tile([128, S], F32, tag="g", name="h32")
                nc.scalar.activation(h32[:], ps[:], ACT.Relu,
                                     bias=bt[:, 32 + i:33 + i])
                hr = gpool.tile([128, S], F32R, tag="g", name="hr")
                nc.vector.tensor_copy(hr[:], h32[:])
                hid.append(hr)
            onats = []
            for gidx in range(S // 128):
                onat = opool.tile([128, H], F32, tag="o", name=f"onat{gidx}")
                onats.append(onat)
            for i in range(4):
                ps = xpps.tile([128, S], F32, tag="xp", name="ops")
                for k in range(4):
                    nc.tensor.matmul(ps[:], w2[:, k, i * 128:(i + 1) * 128],
                                     hid[k][:], start=k == 0, stop=k == 3)
                o32 = gpool.tile([128, S], F32, tag="g", name="o32")
                nc.vector.tensor_scalar_add(o32[:], ps[:], bt[:, 36 + i:37 + i])
                for gidx in range(S // 128):
                    tp = ghps.tile([128, 128], F32, tag="gh", name="tp")
                    nc.tensor.transpose(tp[:], o32[:, gidx * 128:(gidx + 1) * 128],
                                        ident[:])
                    nc.vector.tensor_copy(onats[gidx][:, i * 128:(i + 1) * 128],
                                          tp[:])
            for gidx in range(S // 128):
                r0 = t * S + gidx * 128
                nc.sync.dma_start(y_d[r0:r0 + 128, :], onats[gidx][:])

        for t in range(ntiles):
            emit_mlp(t, hfs[t], hbs[t])

    nc.compile()
    return nc


def kernel(padded_window, window_len, Wih_f, Whh_f, bih_f, bhh_f,
           Wih_b, Whh_b, bih_b, bhh_b, W1, b1, W2, b2):
    wl = np.asarray(window_len)
    lf = (wl - 1) // 2 + 1
    lb = wl // 2 + 1
    order = np.argsort(wl, kind="stable")

    Bc = B // NCORES
    ntiles = Bc // S
    lf_pc = lf[order].reshape(-1, NCORES)
    lb_pc = lb[order].reshape(-1, NCORES)

    def r64(v):
        return int(min(S, -(-int(v) // 64) * 64))

    mode = os.environ.get("GRU_SCHED", "exact")

    def dir_steps(lens_pc, t):
        seg = lens_pc[t * S:(t + 1) * S]  # [S, NCORES]
        n = int(seg.max())
        steps = []
        prev_cmax = 0
        prev_wr = 0
        for j in range(n):
            need = n - j
            cnt = (seg >= need).sum(axis=0)
            cmax, cmin = int(cnt.max()), int(cnt.min())
            if mode == "v1ish":
                wr = wz = int(min(S, max(256, r64(cmax))))
                wh = prev_wr if j > 0 else 0
            elif mode == "a":  # exact wr, wz==wr, wide wh
                wr = wz = r64(cmax)
                wh = prev_wr if j > 0 else 0
            elif mode == "b":  # clamped wr, narrow wh
                wr = wz = int(min(S, max(256, r64(cmax))))
                wh = r64(prev_cmax) if j > 0 else 0
            else:
                wr = r64(cmax)
                wz = wr if wr <= 128 else max(256, wr)
                if mode == "nowh":
                    wh = prev_wr if j > 0 else 0
                else:
                    wh = r64(prev_cmax) if j > 0 else 0
            wmask = wr - cmin  # width of the over-included (to-freeze) prefix
            steps.append((wr, wz, wh, wmask))
            prev_cmax = cmax
            prev_wr = wr
        return tuple(steps)

    sched = tuple((dir_steps(lf_pc, t), dir_steps(lb_pc, t))
                  for t in range(ntiles))

    cache_key = (sched, bool(os.environ.get("GRU_DBG")))
    if cache_key not in _PROGRAM_CACHE:
        _PROGRAM_CACHE[cache_key] = _build_program(sched)
    nc = _PROGRAM_CACHE[cache_key]

    f32 = np.float32
    WihfT, WhhfT = Wih_f.T.astype(f32), Whh_f.T.astype(f32)
    WihbT, WhhbT = Wih_b.T.astype(f32), Whh_b.T.astype(f32)
    wzn_f = np.ascontiguousarray(WihfT[:, H:] * SW)
    wzn_b = np.ascontiguousarray(WihbT[:, H:] * SW)
    w8i_f = (WihfT[:, :H] * SW).astype(E4NP)
    w8i_b = (WihbT[:, :H] * SW).astype(E4NP)
    w8h_f = (WhhfT * SW).astype(E4NP)
    w8h_b = (WhhbT * SW).astype(E4NP)
    w1 = np.ascontiguousarray(W1.T, dtype=f32)
    w2 = np.ascontiguousarray(W2.T, dtype=f32)

    def chunks(v):  # [512] -> [4, 128]
        return np.asarray(v, f32).reshape(4, 128)

    bias = np.concatenate([
        chunks((bih_f + bhh_f)[:H]), chunks(-(bih_f + bhh_f)[H:2 * H]),
        chunks(SW * bhh_f[2 * H:]), chunks(bih_f[2 * H:]),
        chunks((bih_b + bhh_b)[:H]), chunks(-(bih_b + bhh_b)[H:2 * H]),
        chunks(SW * bhh_b[2 * H:]), chunks(bih_b[2 * H:]),
        chunks(b1), chunks(b2),
    ], 0)  # [40, 128]

    pw = np.asarray(padded_window, f32)
    in_maps = []
    p8 = np.arange(8)
    for c in range(NCORES):
        idx = order[c::NCORES]
        xT = np.ascontiguousarray(pw[idx].transpose(1, 2, 0))  # [15, 512, Bc]
        mzf = (BIGM * (p8[:, None] < (8 - lf[idx])[None, :])).astype(f32)
        mzb = (BIGM * (p8[:, None] < (8 - lb[idx])[None, :])).astype(f32)
        in_maps.append({
            "xT": xT, "xT8": xT.astype(E4NP),
            "wznf": wzn_f, "wznb": wzn_b,
            "w8if": w8i_f, "w8ib": w8i_b, "w8hf": w8h_f, "w8hb": w8h_b,
            "w1": w1, "w2": w2,
            "bias": bias, "maskzf": mzf, "maskzb": mzb,
        })

    trace = bool(os.environ.get("GRU_TRACE"))
    kw = {}
    if os.environ.get("GRU_TMPDIR"):
        kw["tmpdir"] = os.environ["GRU_TMPDIR"]
    res = run_bass_kernel_spmd(nc, in_maps, core_ids=list(range(NCORES)),
                               trace=trace, **kw)
    global LAST_RESULT
    LAST_RESULT = res
    out = np.empty((B, H), f32)
    for c in range(NCORES):
        out[order[c::NCORES]] = res.results[c]["y"]
    return out


# revision 22
# speedup vs baseline: 1.0152x; 1.0047x over previous
"""BiGRU encoder kernel for 8 Trainium2 NeuronCores.

Strategy (v2 — mixed fp8/fp32r):
  - Same ragged reformulation as v1: masked GRUs over FIXED position ranges
    (fwd 0..7 ascending, bwd 14..7 descending); sort samples by window_len,
    deal round-robin to 8 cores; per core two batch tiles of 512; each GRU
    step runs on the suffix of samples long enough to need it.
  - Freeze semantics via the UPDATE-gate complement: h' = h + z~*(n - h)
    with z~ = sigmoid(-(pre_z)).  Pre-start samples get +2560 added to the
    raw pre_z, so z~ = sigmoid(-40) ~ 4e-18 and h' rounds back to h exactly
    even in bf16 (v1's h' = n + z*(h-n) would drift in bf16).
  - Tensor-engine mixed precision (sim rel-err 0.93% vs 2e-2 budget):
      * r-gate input+hidden and z/n-gate hidden matmuls: fp8e4m3 with
        MatmulPerfMode.DoubleRow (K=256 per instruction, 2x fp32r rate).
      * z/n-gate input matmuls and the MLP: fp32r (z-input and MLP are the
        accuracy-critical paths; n-input close behind).
    Weights are pre-scaled by 64 host-side (fp8 subnormal avoidance) and the
    1/64 folds into the activation `scale`.
  - fp8 DoubleRow has no 256-wide rate cliff, so fp8 matmuls run at exact
    suffix widths (64-granular); fp32r keeps the >=256 clamp; hidden-side
    matmuls use the PREVIOUS step's count (h==0 for just-started samples,
    and over-included samples are exact because start=True clears the PSUM
    bank and z~=0 freezes them).
  - h carried in bf16 (elementwise chain gets 2x DVE); a per-step fp8 copy
    of h (gpsimd) feeds the hidden-side DoubleRow matmuls; final h in fp32
    for the fp32r MLP.  Mask-add moved to gpsimd.
"""

import os
from contextlib import ExitStack

import numpy as np
import ml_dtypes

import concourse.bacc as bacc
import concourse.tile as tile
from concourse import mybir
from concourse.bass_utils import run_bass_kernel_spmd
from concourse.masks import make_identity

NCORES = 8
B, T, D, H = 8192, 15, 512, 512
G = 3 * H
SW = 64.0          # weight pre-scale
BIGM = 40.0 * SW   # mask value on the 64-scaled pre-activation
S = 512
F32 = mybir.dt.float32
F32R = mybir.dt.float32r
BF16 = mybir.dt.bfloat16
F8 = mybir.dt.float8e4
DR = mybir.MatmulPerfMode.DoubleRow
E4NP = ml_dtypes.float8_e4m3

ACT = mybir.ActivationFunctionType
ALU = mybir.AluOpType

_PROGRAM_CACHE = {}
LAST_RESULT = None


def _build_program(sched):
    """sched: per tile, (f_steps, b_steps); step = (wr, wz, wh, masked)."""
    ntiles = len(sched)
    Bc = S * ntiles
    nc = bacc.Bacc("TRN2", target_bir_lowering=False, debug=False,
                   num_devices=NCORES)

    xT_d = nc.dram_tensor("xT", [T, D, Bc], F32R, kind="ExternalInput")
    x8T_d = nc.dram_tensor("xT8", [T, D, Bc], F8, kind="ExternalInput")
    wzn_f_d = nc.dram_tensor("wznf", [D, 2 * H], F32R, kind="ExternalInput")
    wzn_b_d = nc.dram_tensor("wznb", [D, 2 * H], F32R, kind="ExternalInput")
    w8i_f_d = nc.dram_tensor("w8if", [D, H], F8, kind="ExternalInput")
    w8i_b_d = nc.dram_tensor("w8ib", [D, H], F8, kind="ExternalInput")
    w8h_f_d = nc.dram_tensor("w8hf", [H, G], F8, kind="ExternalInput")
    w8h_b_d = nc.dram_tensor("w8hb", [H, G], F8, kind="ExternalInput")
    w1_d = nc.dram_tensor("w1", [2 * H, H], F32R, kind="ExternalInput")
    w2_d = nc.dram_tensor("w2", [H, H], F32R, kind="ExternalInput")
    bias_d = nc.dram_tensor("bias", [40, 128], F32, kind="ExternalInput")
    mf_d = nc.dram_tensor("maskzf", [8, Bc], F32, kind="ExternalInput")
    mb_d = nc.dram_tensor("maskzb", [8, Bc], F32, kind="ExternalInput")
    y_d = nc.dram_tensor("y", [Bc, H], F32, kind="ExternalOutput")
    dbg = bool(os.environ.get("GRU_DBG"))
    if dbg:
        hf_d = nc.dram_tensor("hfdbg", [ntiles, 128, 4, S], F32,
                              kind="ExternalOutput")
        hb_d = nc.dram_tensor("hbdbg", [ntiles, 128, 4, S], F32,
                              kind="ExternalOutput")

    with tile.TileContext(nc) as tc, ExitStack() as ctx:
        const = ctx.enter_context(tc.tile_pool(name="const", bufs=1))
        wpool = ctx.enter_context(tc.tile_pool(name="w", bufs=2))
        w8pool = ctx.enter_context(tc.tile_pool(name="w8", bufs=4))
        xpool = ctx.enter_context(tc.tile_pool(name="x", bufs=3))
        x8pool = ctx.enter_context(tc.tile_pool(name="x8", bufs=3))
        hpool = ctx.enter_context(tc.tile_pool(name="h", bufs=2))
        h8pool = ctx.enter_context(tc.tile_pool(name="h8", bufs=2))
        hfin = ctx.enter_context(tc.tile_pool(name="hfin", bufs=4))
        gpool = ctx.enter_context(tc.tile_pool(name="g", bufs=6))
        mpool = ctx.enter_context(tc.tile_pool(name="m", bufs=2))
        opool = ctx.enter_context(tc.tile_pool(name="o", bufs=4))
        rzps = ctx.enter_context(tc.tile_pool(name="rz", bufs=4, space="PSUM"))
        xpps = ctx.enter_context(tc.tile_pool(name="xp", bufs=2, space="PSUM"))
        ghps = ctx.enter_context(tc.tile_pool(name="gh", bufs=2, space="PSUM"))

        def load_w(dram, kchunks, cols, name, dt=F32R, pool=None, tag=None,
                   spread=False):
            t_ = (pool or wpool).tile([128, kchunks, cols], dt,
                                      tag=tag or ("w" if pool is None else "const"),
                                      name=name)
            src = dram.rearrange("(c k) g -> k c g", k=128)
            for c in range(kchunks):
                eng = nc.sync if spread and c < 2 else nc.scalar
                eng.dma_start(t_[:, c, :], src[:, c, :])
            return t_

        # fp8 weights first: tiny DMAs, lets the r-gate DR matmuls start early
        w8i_f = load_w(w8i_f_d, 4, H, "w8if", dt=F8, pool=w8pool, tag="w8", spread=True)
        w8h_f = load_w(w8h_f_d, 4, G, "w8hf", dt=F8, pool=w8pool, tag="w8")
        wzn_f = load_w(wzn_f_d, 4, 2 * H, "wznf", spread=True)
        w8i_b = load_w(w8i_b_d, 4, H, "w8ib", dt=F8, pool=w8pool, tag="w8")
        w8h_b = load_w(w8h_b_d, 4, G, "w8hb", dt=F8, pool=w8pool, tag="w8")
        wzn_b = load_w(wzn_b_d, 4, 2 * H, "wznb", spread=True)
        w2 = load_w(w2_d, 4, H, "w2", pool=const)
        bt = const.tile([128, 40], F32)
        nc.gpsimd.dma_start(bt[:], bias_d.rearrange("n p -> p n"))
        ident = const.tile([128, 128], F32)
        make_identity(nc, ident[:])

        def emit_dir(s0, steps, wzn, w8i, w8h, mask_d, bb, pos_fn):
            """One GRU direction over one batch tile; returns final h tile."""
            nsteps = len(steps)
            h_prev = None
            h8_prev = None
            for j, (wr, wz, wh, wmask) in enumerate(steps):
                first = j == 0
                last = j == nsteps - 1
                masked = wmask > 0
                p_abs = pos_fn(j)
                so = S - wr    # chain/suffix offset
                soz = S - wz
                soh = S - wh if wh else S
                x8 = x8pool.tile([128, 4, S], F8, tag="x8", name="x8")
                nc.sync.dma_start(
                    x8[:, :, so:],
                    x8T_d[p_abs].rearrange("(c k) s -> k c s", k=128)[:, :, s0 + so:s0 + S],
                )
                xt = xpool.tile([128, 4, S], F32R, tag="x", name="xt")
                nc.sync.dma_start(
                    xt[:, :, soz:],
                    xT_d[p_abs].rearrange("(c k) s -> k c s", k=128)[:, :, s0 + soz:s0 + S],
                )
                mt = None
                if masked:
                    mt = mpool.tile([128, S], F32, tag="m", name="mt")
                    nc.gpsimd.dma_start(
                        mt[:, :wmask],
                        mask_d[8 - nsteps + j,
                               s0 + so:s0 + so + wmask].partition_broadcast(128),
                    )
                h_next = (hfin if last else hpool).tile(
                    [128, 4, S], F32R if last else BF16,
                    tag="hf" if last else "h", name="h")
                h8_next = None
                if not last:
                    h8_next = h8pool.tile([128, 4, S], F8, tag="h8", name="h8")
                    nwr = steps[j + 1][0]
                    if S - nwr < so:  # next step includes more samples
                        meng = nc.vector if os.environ.get("GRU_MS") == "v" else nc.gpsimd
                        meng.memset(h_next[:, :, S - nwr:so], 0.0)

                for i in range(4):
                    r_ps = rzps.tile([128, wr], F32, tag="rz", name=f"rps{i}")
                    z_ps = rzps.tile([128, wz], F32, tag="rz", name=f"zps{i}")
                    xpn = xpps.tile([128, wz], F32, tag="xp", name=f"xpn{i}")
                    # r-gate input: fp8 DoubleRow over 2 k-pairs
                    for p in range(2):
                        nc.tensor.matmul(
                            r_ps[:], w8i[:, 2 * p:2 * p + 2, i * 128:(i + 1) * 128],
                            x8[:, 2 * p:2 * p + 2, so:],
                            start=p == 0, stop=first and p == 1, perf_mode=DR)
                    # z/n input: fp32r
                    for k in range(4):
                        nc.tensor.matmul(z_ps[:], wzn[:, k, i * 128:(i + 1) * 128],
                                         xt[:, k, soz:], start=k == 0,
                                         stop=first and k == 3)
                        nc.tensor.matmul(xpn[:],
                                         wzn[:, k, H + i * 128:H + (i + 1) * 128],
                                         xt[:, k, soz:], start=k == 0, stop=k == 3)
                    ghn = None
                    if not first:
                        ghn = ghps.tile([128, wh], F32, tag="gh", name=f"ghn{i}")
                        for p in range(2):
                            hs = h8_prev[:, 2 * p:2 * p + 2, soh:]
                            nc.tensor.matmul(
                                r_ps[:, wr - wh:],
                                w8h[:, 2 * p:2 * p + 2, i * 128:(i + 1) * 128],
                                hs, start=False, stop=p == 1, perf_mode=DR)
                            nc.tensor.matmul(
                                z_ps[:, wz - wh:],
                                w8h[:, 2 * p:2 * p + 2, H + i * 128:H + (i + 1) * 128],
                                hs, start=False, stop=p == 1, perf_mode=DR)
                            nc.tensor.matmul(
                                ghn[:],
                                w8h[:, 2 * p:2 * p + 2, 2 * H + i * 128:2 * H + (i + 1) * 128],
                                hs, start=p == 0, stop=p == 1, perf_mode=DR)

                    r = gpool.tile([128, wr], BF16, tag="g", name="r")
                    nc.scalar.activation(r[:], r_ps[:], ACT.Sigmoid,
                                         bias=bt[:, bb + i:bb + i + 1],
                                         scale=1.0 / SW)
                    if masked:
                        # freeze the over-included prefix: in-place +2560 on
                        # the 64-scaled z pre-activation (-> z~ = sigmoid(-40))
                        zp = z_ps[:, wz - wr:wz - wr + wmask]
                        if os.environ.get("GRU_MA") == "t":
                            zin = gpool.tile([128, wmask], F32, tag="g", name="zin")
                            nc.vector.tensor_add(zin[:], zp, mt[:, :wmask])
                            nc.vector.tensor_copy(zp, zin[:])
                        else:
                            nc.vector.tensor_add(zp, zp, mt[:, :wmask])
                    z = gpool.tile([128, wr], BF16, tag="g", name="z")
                    nc.scalar.activation(z[:], z_ps[:, wz - wr:], ACT.Sigmoid,
                                         bias=bt[:, bb + 4 + i:bb + 5 + i],
                                         scale=-1.0 / SW)
                    tt = gpool.tile([128, wr], BF16, tag="g", name="tt")
                    if first:
                        nc.vector.tensor_scalar_mul(tt[:], r[:],
                                                    bt[:, bb + 8 + i:bb + 9 + i])
                    else:
                        if wh < wr:
                            # just-started samples: h_prev == 0, so the hidden
                            # n-term is exactly the bhh_n bias
                            nc.vector.tensor_scalar_mul(
                                tt[:, :wr - wh], r[:, :wr - wh],
                                bt[:, bb + 8 + i:bb + 9 + i])
                        nc.vector.scalar_tensor_tensor(
                            tt[:, wr - wh:], ghn[:],
                            bt[:, bb + 8 + i:bb + 9 + i], r[:, wr - wh:],
                            op0=ALU.add, op1=ALU.mult)
                    ss = gpool.tile([128, wr], BF16, tag="g", name="ss")
                    nc.vector.tensor_add(ss[:], tt[:], xpn[:, wz - wr:])
                    n = gpool.tile([128, wr], BF16, tag="g", name="n")
                    nc.scalar.activation(n[:], ss[:], ACT.Tanh,
                                         bias=bt[:, bb + 12 + i:bb + 13 + i],
                                         scale=1.0 / SW)
                    ho = h_next[:, i, so:]
                    if first:
                        nc.vector.tensor_mul(ho, z[:], n[:])
                    else:
                        dd = gpool.tile([128, wr], BF16, tag="g", name="dd")
                        nc.vector.tensor_sub(dd[:], n[:], h_prev[:, i, so:])
                        e = gpool.tile([128, wr], BF16, tag="g", name="e")
                        nc.vector.tensor_mul(e[:], z[:], dd[:])
                        nc.vector.tensor_add(ho, h_prev[:, i, so:], e[:])
                    if not last:
                        # fp8 copy for next step's hidden matmuls; scalar engine
                        # casts ~5x faster than gpsimd and is off-critical here
                        nc.scalar.activation(h8_next[:, i, so:], ho, ACT.Copy)
                h_prev = h_next
                h8_prev = h8_next
            return h_prev

        hfs = []
        for t in range(ntiles):
            nf = len(sched[t][0])
            hfs.append(emit_dir(t * S, sched[t][0], wzn_f, w8i_f, w8h_f, mf_d,
                                0, lambda j, nf=nf: 8 - nf + j))
            if dbg:
                nc.sync.dma_start(hf_d[t], hfs[t][:].bitcast(F32))
        hbs = []
        for t in range(ntiles):
            nb = len(sched[t][1])
            hbs.append(emit_dir(t * S, sched[t][1], wzn_b, w8i_b, w8h_b, mb_d,
                                16, lambda j, nb=nb: 6 + nb - j))
            if dbg:
                nc.sync.dma_start(hb_d[t], hbs[t][:].bitcast(F32))
        w1 = load_w(w1_d, 8, H, "w1")

        def emit_mlp(t, hf, hb):
            hid = []
            for i in range(4):
                ps = xpps.tile([128, S], F32, tag="xp", name="mps")
                for k in range(8):
                    src = hf if k < 4 else hb
                    nc.tensor.matmul(ps[:], w1[:, k, i * 128:(i + 1) * 128],
                                     src[:, k % 4, :], start=k == 0, stop=k == 7)
                h32 = gpool.tile([128, S], F32, tag="g", name="h32")
                nc.scalar.activation(h32[:], ps[:], ACT.Relu,
                                     bias=bt[:, 32 + i:33 + i])
                hr = gpool.tile([128, S], F32R, tag="g", name="hr")
                nc.vector.tensor_copy(hr[:], h32[:])
                hid.append(hr)
            onats = []
            for gidx in range(S // 128):
                onat = opool.tile([128, H], F32, tag="o", name=f"onat{gidx}")
                onats.append(onat)
            for i in range(4):
                ps = xpps.tile([128, S], F32, tag="xp", name="ops")
                for k in range(4):
                    nc.tensor.matmul(ps[:], w2[:, k, i * 128:(i + 1) * 128],
                                     hid[k][:], start=k == 0, stop=k == 3)
                o32 = gpool.tile([128, S], F32, tag="g", name="o32")
                nc.vector.tensor_scalar_add(o32[:], ps[:], bt[:, 36 + i:37 + i])
                for gidx in range(S // 128):
                    tp = ghps.tile([128, 128], F32, tag="gh", name="tp")
                    nc.tensor.transpose(tp[:], o32[:, gidx * 128:(gidx + 1) * 128],
                                        ident[:])
                    nc.vector.tensor_copy(onats[gidx][:, i * 128:(i + 1) * 128],
                                          tp[:])
            for gidx in range(S // 128):
                r0 = t * S + gidx * 128
                nc.sync.dma_start(y_d[r0:r0 + 128, :], onats[gidx][:])

        for t in range(ntiles):
            emit_mlp(t, hfs[t], hbs[t])

    nc.compile()
    return nc


def kernel(padded_window, window_len, Wih_f, Whh_f, bih_f, bhh_f,
           Wih_b, Whh_b, bih_b, bhh_b, W1, b1, W2, b2):
    wl = np.asarray(window_len)
    lf = (wl - 1) // 2 + 1
    lb = wl // 2 + 1
    order = np.argsort(wl, kind="stable")

    Bc = B // NCORES
    ntiles = Bc // S
    lf_pc = lf[order].reshape(-1, NCORES)
    lb_pc = lb[order].reshape(-1, NCORES)

    def r64(v):
        return int(min(S, -(-int(v) // 64) * 64))

    mode = os.environ.get("GRU_SCHED", "exact")

    def dir_steps(lens_pc, t):
        seg = lens_pc[t * S:(t + 1) * S]  # [S, NCORES]
        n = int(seg.max())
        steps = []
        prev_cmax = 0
        prev_wr = 0
        for j in range(n):
            need = n - j
            cnt = (seg >= need).sum(axis=0)
            cmax, cmin = int(cnt.max()), int(cnt.min())
            if mode == "v1ish":
                wr = wz = int(min(S, max(256, r64(cmax))))
                wh = prev_wr if j > 0 else 0
            elif mode == "a":  # exact wr, wz==wr, wide wh
                wr = wz = r64(cmax)
                wh = prev_wr if j > 0 else 0
            elif mode == "b":  # clamped wr, narrow wh
                wr = wz = int(min(S, max(256, r64(cmax))))
                wh = r64(prev_cmax) if j > 0 else 0
            else:
                wr = r64(cmax)
                wz = wr if wr <= 128 else max(256, wr)
                if mode == "nowh":
                    wh = prev_wr if j > 0 else 0
                else:
                    wh = r64(prev_cmax) if j > 0 else 0
            wmask = wr - cmin  # width of the over-included (to-freeze) prefix
            steps.append((wr, wz, wh, wmask))
            prev_cmax = cmax
            prev_wr = wr
        return tuple(steps)

    sched = tuple((dir_steps(lf_pc, t), dir_steps(lb_pc, t))
                  for t in range(ntiles))

    cache_key = (sched, bool(os.environ.get("GRU_DBG")))
    if cache_key not in _PROGRAM_CACHE:
        _PROGRAM_CACHE[cache_key] = _build_program(sched)
    nc = _PROGRAM_CACHE[cache_key]

    f32 = np.float32
    WihfT, WhhfT = Wih_f.T.astype(f32), Whh_f.T.astype(f32)
    WihbT, WhhbT = Wih_b.T.astype(f32), Whh_b.T.astype(f32)
    wzn_f = np.ascontiguousarray(WihfT[:, H:] * SW)
    wzn_b = np.ascontiguousarray(WihbT[:, H:] * SW)
    w8i_f = (WihfT[:, :H] * SW).astype(E4NP)
    w8i_b = (WihbT[:, :H] * SW).astype(E4NP)
    w8h_f = (WhhfT * SW).astype(E4NP)
    w8h_b = (WhhbT * SW).astype(E4NP)
    w1 = np.ascontiguousarray(W1.T, dtype=f32)
    w2 = np.ascontiguousarray(W2.T, dtype=f32)

    def chunks(v):  # [512] -> [4, 128]
        return np.asarray(v, f32).reshape(4, 128)

    bias = np.concatenate([
        chunks((bih_f + bhh_f)[:H]), chunks(-(bih_f + bhh_f)[H:2 * H]),
        chunks(SW * bhh_f[2 * H:]), chunks(bih_f[2 * H:]),
        chunks((bih_b + bhh_b)[:H]), chunks(-(bih_b + bhh_b)[H:2 * H]),
        chunks(SW * bhh_b[2 * H:]), chunks(bih_b[2 * H:]),
        chunks(b1), chunks(b2),
    ], 0)  # [40, 128]

    pw = np.asarray(padded_window, f32)
    in_maps = []
    p8 = np.arange(8)
    for c in range(NCORES):
        idx = order[c::NCORES]
        xT = np.ascontiguousarray(pw[idx].transpose(1, 2, 0))  # [15, 512, Bc]
        mzf = (BIGM * (p8[:, None] < (8 - lf[idx])[None, :])).astype(f32)
        mzb = (BIGM * (p8[:, None] < (8 - lb[idx])[None, :])).astype(f32)
        in_maps.append({
            "xT": xT, "xT8": xT.astype(E4NP),
            "wznf": wzn_f, "wznb": wzn_b,
            "w8if": w8i_f, "w8ib": w8i_b, "w8hf": w8h_f, "w8hb": w8h_b,
            "w1": w1, "w2": w2,
            "bias": bias, "maskzf": mzf, "maskzb": mzb,
        })

    trace = bool(os.environ.get("GRU_TRACE"))
    kw = {}
    if os.environ.get("GRU_TMPDIR"):
        kw["tmpdir"] = os.environ["GRU_TMPDIR"]
    res = run_bass_kernel_spmd(nc, in_maps, core_ids=list(range(NCORES)),
                               trace=trace, **kw)
    global LAST_RESULT
    LAST_RESULT = res
    out = np.empty((B, H), f32)
    for c in range(NCORES):
        out[order[c::NCORES]] = res.results[c]["y"]
    return out


# revision 23
# speedup vs baseline: 1.0371x; 1.0215x over previous
"""BiGRU encoder kernel for 8 Trainium2 NeuronCores.

Strategy (v2 — mixed fp8/fp32r):
  - Same ragged reformulation as v1: masked GRUs over FIXED position ranges
    (fwd 0..7 ascending, bwd 14..7 descending); sort samples by window_len,
    deal round-robin to 8 cores; per core two batch tiles of 512; each GRU
    step runs on the suffix of samples long enough to need it.
  - Freeze semantics via the UPDATE-gate complement: h' = h + z~*(n - h)
    with z~ = sigmoid(-(pre_z)).  Pre-start samples get +2560 added to the
    raw pre_z, so z~ = sigmoid(-40) ~ 4e-18 and h' rounds back to h exactly
    even in bf16 (v1's h' = n + z*(h-n) would drift in bf16).
  - Tensor-engine mixed precision (sim rel-err 0.93% vs 2e-2 budget):
      * r-gate input+hidden and z/n-gate hidden matmuls: fp8e4m3 with
        MatmulPerfMode.DoubleRow (K=256 per instruction, 2x fp32r rate).
      * z/n-gate input matmuls and the MLP: fp32r (z-input and MLP are the
        accuracy-critical paths; n-input close behind).
    Weights are pre-scaled by 64 host-side (fp8 subnormal avoidance) and the
    1/64 folds into the activation `scale`.
  - fp8 DoubleRow has no 256-wide rate cliff, so fp8 matmuls run at exact
    suffix widths (64-granular); fp32r keeps the >=256 clamp; hidden-side
    matmuls use the PREVIOUS step's count (h==0 for just-started samples,
    and over-included samples are exact because start=True clears the PSUM
    bank and z~=0 freezes them).
  - h carried in bf16 (elementwise chain gets 2x DVE); a per-step fp8 copy
    of h (gpsimd) feeds the hidden-side DoubleRow matmuls; final h in fp32
    for the fp32r MLP.  Mask-add moved to gpsimd.
"""

import os
from contextlib import ExitStack

import numpy as np
import ml_dtypes

import concourse.bacc as bacc
import concourse.tile as tile
from concourse import mybir
from concourse.bass_utils import run_bass_kernel_spmd
from concourse.masks import make_identity

NCORES = 8
B, T, D, H = 8192, 15, 512, 512
G = 3 * H
SW = 64.0          # weight pre-scale
BIGM = 40.0 * SW   # mask value on the 64-scaled pre-activation
S = 512
F32 = mybir.dt.float32
F32R = mybir.dt.float32r
BF16 = mybir.dt.bfloat16
F8 = mybir.dt.float8e4
DR = mybir.MatmulPerfMode.DoubleRow
E4NP = ml_dtypes.float8_e4m3

ACT = mybir.ActivationFunctionType
ALU = mybir.AluOpType

_PROGRAM_CACHE = {}
LAST_RESULT = None


def _build_program(sched):
    """sched: per tile, (f_steps, b_steps); step = (wr, wz, wh, masked)."""
    ntiles = len(sched)
    Bc = S * ntiles
    nc = bacc.Bacc("TRN2", target_bir_lowering=False, debug=False,
                   num_devices=NCORES)

    xT_d = nc.dram_tensor("xT", [T, D, Bc], F32R, kind="ExternalInput")
    x8T_d = nc.dram_tensor("xT8", [T, D, Bc], F8, kind="ExternalInput")
    wn_f_d = nc.dram_tensor("wnf", [D, H], F32R, kind="ExternalInput")
    wn_b_d = nc.dram_tensor("wnb", [D, H], F32R, kind="ExternalInput")
    w8i_f_d = nc.dram_tensor("w8if", [D, 2 * H], F8, kind="ExternalInput")
    w8i_b_d = nc.dram_tensor("w8ib", [D, 2 * H], F8, kind="ExternalInput")
    w8h_f_d = nc.dram_tensor("w8hf", [H, G], F8, kind="ExternalInput")
    w8h_b_d = nc.dram_tensor("w8hb", [H, G], F8, kind="ExternalInput")
    w1_d = nc.dram_tensor("w1", [2 * H, H], F32R, kind="ExternalInput")
    w2_d = nc.dram_tensor("w2", [H, H], F32R, kind="ExternalInput")
    bias_d = nc.dram_tensor("bias", [40, 128], F32, kind="ExternalInput")
    mf_d = nc.dram_tensor("maskzf", [8, Bc], F32, kind="ExternalInput")
    mb_d = nc.dram_tensor("maskzb", [8, Bc], F32, kind="ExternalInput")
    y_d = nc.dram_tensor("y", [Bc, H], F32, kind="ExternalOutput")
    dbg = bool(os.environ.get("GRU_DBG"))
    if dbg:
        hf_d = nc.dram_tensor("hfdbg", [ntiles, 128, 4, S], F32,
                              kind="ExternalOutput")
        hb_d = nc.dram_tensor("hbdbg", [ntiles, 128, 4, S], F32,
                              kind="ExternalOutput")

    with tile.TileContext(nc) as tc, ExitStack() as ctx:
        const = ctx.enter_context(tc.tile_pool(name="const", bufs=1))
        wpool = ctx.enter_context(tc.tile_pool(name="w", bufs=2))
        w8pool = ctx.enter_context(tc.tile_pool(name="w8", bufs=4))
        xpool = ctx.enter_context(tc.tile_pool(name="x", bufs=3))
        x8pool = ctx.enter_context(tc.tile_pool(name="x8", bufs=3))
        hpool = ctx.enter_context(tc.tile_pool(name="h", bufs=2))
        h8pool = ctx.enter_context(tc.tile_pool(name="h8", bufs=2))
        hfin = ctx.enter_context(tc.tile_pool(name="hfin", bufs=4))
        gpool = ctx.enter_context(tc.tile_pool(name="g", bufs=6))
        mpool = ctx.enter_context(tc.tile_pool(name="m", bufs=2))
        opool = ctx.enter_context(tc.tile_pool(name="o", bufs=4))
        rzps = ctx.enter_context(tc.tile_pool(name="rz", bufs=4, space="PSUM"))
        xpps = ctx.enter_context(tc.tile_pool(name="xp", bufs=2, space="PSUM"))
        ghps = ctx.enter_context(tc.tile_pool(name="gh", bufs=2, space="PSUM"))

        def load_w(dram, kchunks, cols, name, dt=F32R, pool=None, tag=None,
                   spread=False):
            t_ = (pool or wpool).tile([128, kchunks, cols], dt,
                                      tag=tag or ("w" if pool is None else "const"),
                                      name=name)
            src = dram.rearrange("(c k) g -> k c g", k=128)
            for c in range(kchunks):
                eng = nc.sync if spread and c < 2 else nc.scalar
                eng.dma_start(t_[:, c, :], src[:, c, :])
            return t_

        # fp8 weights first: tiny DMAs, lets the r/z-gate DR matmuls start early
        w8i_f = load_w(w8i_f_d, 4, 2 * H, "w8if", dt=F8, pool=w8pool, tag="w8",
                       spread=True)
        w8h_f = load_w(w8h_f_d, 4, G, "w8hf", dt=F8, pool=w8pool, tag="w8")
        wn_f = load_w(wn_f_d, 4, H, "wnf", spread=True)
        w8i_b = load_w(w8i_b_d, 4, 2 * H, "w8ib", dt=F8, pool=w8pool, tag="w8")
        w8h_b = load_w(w8h_b_d, 4, G, "w8hb", dt=F8, pool=w8pool, tag="w8")
        wn_b = load_w(wn_b_d, 4, H, "wnb", spread=True)
        w2 = load_w(w2_d, 4, H, "w2", pool=const)
        bt = const.tile([128, 40], F32)
        nc.gpsimd.dma_start(bt[:], bias_d.rearrange("n p -> p n"))
        ident = const.tile([128, 128], F32)
        make_identity(nc, ident[:])

        def emit_dir(s0, steps, wn, w8i, w8h, mask_d, bb, pos_fn):
            """One GRU direction over one batch tile; returns final h tile."""
            nsteps = len(steps)
            h_prev = None
            h8_prev = None
            for j, (wr, wz, wh, wmask) in enumerate(steps):
                first = j == 0
                last = j == nsteps - 1
                masked = wmask > 0
                p_abs = pos_fn(j)
                so = S - wr    # chain/suffix offset
                soz = S - wz
                soh = S - wh if wh else S
                x8 = x8pool.tile([128, 4, S], F8, tag="x8", name="x8")
                nc.sync.dma_start(
                    x8[:, :, so:],
                    x8T_d[p_abs].rearrange("(c k) s -> k c s", k=128)[:, :, s0 + so:s0 + S],
                )
                xt = xpool.tile([128, 4, S], F32R, tag="x", name="xt")
                nc.sync.dma_start(
                    xt[:, :, soz:],
                    xT_d[p_abs].rearrange("(c k) s -> k c s", k=128)[:, :, s0 + soz:s0 + S],
                )
                mt = None
                if masked:
                    mt = mpool.tile([128, S], F32, tag="m", name="mt")
                    nc.gpsimd.dma_start(
                        mt[:, :wmask],
                        mask_d[8 - nsteps + j,
                               s0 + so:s0 + so + wmask].partition_broadcast(128),
                    )
                h_next = (hfin if last else hpool).tile(
                    [128, 4, S], F32R if last else BF16,
                    tag="hf" if last else "h", name="h")
                h8_next = None
                if not last:
                    h8_next = h8pool.tile([128, 4, S], F8, tag="h8", name="h8")
                    nwr = steps[j + 1][0]
                    if S - nwr < so:  # next step includes more samples
                        meng = nc.vector if os.environ.get("GRU_MS") == "v" else nc.gpsimd
                        meng.memset(h_next[:, :, S - nwr:so], 0.0)

                for i in range(4):
                    r_ps = rzps.tile([128, wr], F32, tag="rz", name=f"rps{i}")
                    z_ps = rzps.tile([128, wr], F32, tag="rz", name=f"zps{i}")
                    xpn = xpps.tile([128, wz], F32, tag="xp", name=f"xpn{i}")
                    # r/z-gate input: fp8 DoubleRow over 2 k-pairs
                    for p in range(2):
                        xs = x8[:, 2 * p:2 * p + 2, so:]
                        nc.tensor.matmul(
                            r_ps[:], w8i[:, 2 * p:2 * p + 2, i * 128:(i + 1) * 128],
                            xs, start=p == 0, stop=first and p == 1, perf_mode=DR)
                        nc.tensor.matmul(
                            z_ps[:], w8i[:, 2 * p:2 * p + 2, H + i * 128:H + (i + 1) * 128],
                            xs, start=p == 0, stop=first and p == 1, perf_mode=DR)
                    # n input: fp32r
                    for k in range(4):
                        nc.tensor.matmul(xpn[:],
                                         wn[:, k, i * 128:(i + 1) * 128],
                                         xt[:, k, soz:], start=k == 0, stop=k == 3)
                    ghn = None
                    if not first:
                        ghn = ghps.tile([128, wh], F32, tag="gh", name=f"ghn{i}")
                        for p in range(2):
                            hs = h8_prev[:, 2 * p:2 * p + 2, soh:]
                            nc.tensor.matmul(
                                r_ps[:, wr - wh:],
                                w8h[:, 2 * p:2 * p + 2, i * 128:(i + 1) * 128],
                                hs, start=False, stop=p == 1, perf_mode=DR)
                            nc.tensor.matmul(
                                z_ps[:, wr - wh:],
                                w8h[:, 2 * p:2 * p + 2, H + i * 128:H + (i + 1) * 128],
                                hs, start=False, stop=p == 1, perf_mode=DR)
                            nc.tensor.matmul(
                                ghn[:],
                                w8h[:, 2 * p:2 * p + 2, 2 * H + i * 128:2 * H + (i + 1) * 128],
                                hs, start=p == 0, stop=p == 1, perf_mode=DR)

                    r = gpool.tile([128, wr], BF16, tag="g", name="r")
                    nc.scalar.activation(r[:], r_ps[:], ACT.Sigmoid,
                                         bias=bt[:, bb + i:bb + i + 1],
                                         scale=1.0 / SW)
                    if masked:
                        # freeze the over-included prefix: in-place +2560 on
                        # the 64-scaled z pre-activation (-> z~ = sigmoid(-40))
                        zp = z_ps[:, :wmask]
                        nc.vector.tensor_add(zp, zp, mt[:, :wmask])
                    z = gpool.tile([128, wr], BF16, tag="g", name="z")
                    nc.scalar.activation(z[:], z_ps[:], ACT.Sigmoid,
                                         bias=bt[:, bb + 4 + i:bb + 5 + i],
                                         scale=-1.0 / SW)
                    tt = gpool.tile([128, wr], BF16, tag="g", name="tt")
                    if first:
                        nc.vector.tensor_scalar_mul(tt[:], r[:],
                                                    bt[:, bb + 8 + i:bb + 9 + i])
                    else:
                        if wh < wr:
                            # just-started samples: h_prev == 0, so the hidden
                            # n-term is exactly the bhh_n bias
                            nc.vector.tensor_scalar_mul(
                                tt[:, :wr - wh], r[:, :wr - wh],
                                bt[:, bb + 8 + i:bb + 9 + i])
                        nc.vector.scalar_tensor_tensor(
                            tt[:, wr - wh:], ghn[:],
                            bt[:, bb + 8 + i:bb + 9 + i], r[:, wr - wh:],
                            op0=ALU.add, op1=ALU.mult)
                    ss = gpool.tile([128, wr], BF16, tag="g", name="ss")
                    nc.vector.tensor_add(ss[:], tt[:], xpn[:, wz - wr:])
                    n = gpool.tile([128, wr], BF16, tag="g", name="n")
                    nc.scalar.activation(n[:], ss[:], ACT.Tanh,
                                         bias=bt[:, bb + 12 + i:bb + 13 + i],
                                         scale=1.0 / SW)
                    ho = h_next[:, i, so:]
                    if first:
                        nc.vector.tensor_mul(ho, z[:], n[:])
                    else:
                        dd = gpool.tile([128, wr], BF16, tag="g", name="dd")
                        nc.vector.tensor_sub(dd[:], n[:], h_prev[:, i, so:])
                        e = gpool.tile([128, wr], BF16, tag="g", name="e")
                        nc.vector.tensor_mul(e[:], z[:], dd[:])
                        nc.vector.tensor_add(ho, h_prev[:, i, so:], e[:])
                    if not last:
                        # fp8 copy for next step's hidden matmuls; scalar engine
                        # casts ~5x faster than gpsimd and is off-critical here
                        nc.scalar.activation(h8_next[:, i, so:], ho, ACT.Copy)
                h_prev = h_next
                h8_prev = h8_next
            return h_prev

        hfs = []
        for t in range(ntiles):
            nf = len(sched[t][0])
            hfs.append(emit_dir(t * S, sched[t][0], wn_f, w8i_f, w8h_f, mf_d,
                                0, lambda j, nf=nf: 8 - nf + j))
            if dbg:
                nc.sync.dma_start(hf_d[t], hfs[t][:].bitcast(F32))
        hbs = []
        for t in range(ntiles):
            nb = len(sched[t][1])
            hbs.append(emit_dir(t * S, sched[t][1], wn_b, w8i_b, w8h_b, mb_d,
                                16, lambda j, nb=nb: 6 + nb - j))
            if dbg:
                nc.sync.dma_start(hb_d[t], hbs[t][:].bitcast(F32))
        w1 = load_w(w1_d, 8, H, "w1")

        def emit_mlp(t, hf, hb):
            hid = []
            for i in range(4):
                ps = xpps.tile([128, S], F32, tag="xp", name="mps")
                for k in range(8):
                    src = hf if k < 4 else hb
                    nc.tensor.matmul(ps[:], w1[:, k, i * 128:(i + 1) * 128],
                                     src[:, k % 4, :], start=k == 0, stop=k == 7)
                h32 = gpool.tile([128, S], F32, tag="g", name="h32")
                nc.scalar.activation(h32[:], ps[:], ACT.Relu,
                                     bias=bt[:, 32 + i:33 + i])
                hr = gpool.tile([128, S], F32R, tag="g", name="hr")
                nc.vector.tensor_copy(hr[:], h32[:])
                hid.append(hr)
            onats = []
            for gidx in range(S // 128):
                onat = opool.tile([128, H], F32, tag="o", name=f"onat{gidx}")
                onats.append(onat)
            for i in range(4):
                ps = xpps.tile([128, S], F32, tag="xp", name="ops")
                for k in range(4):
                    nc.tensor.matmul(ps[:], w2[:, k, i * 128:(i + 1) * 128],
                                     hid[k][:], start=k == 0, stop=k == 3)
                o32 = gpool.tile([128, S], F32, tag="g", name="o32")
                nc.vector.tensor_scalar_add(o32[:], ps[:], bt[:, 36 + i:37 + i])
                for gidx in range(S // 128):
                    tp = ghps.tile([128, 128], F32, tag="gh", name="tp")
                    nc.tensor.transpose(tp[:], o32[:, gidx * 128:(gidx + 1) * 128],
                                        ident[:])
                    nc.vector.tensor_copy(onats[gidx][:, i * 128:(i + 1) * 128],
                                          tp[:])
            for gidx in range(S // 128):
                r0 = t * S + gidx * 128
                nc.sync.dma_start(y_d[r0:r0 + 128, :], onats[gidx][:])

        for t in range(ntiles):
            emit_mlp(t, hfs[t], hbs[t])

    nc.compile()
    return nc


def kernel(padded_window, window_len, Wih_f, Whh_f, bih_f, bhh_f,
           Wih_b, Whh_b, bih_b, bhh_b, W1, b1, W2, b2):
    wl = np.asarray(window_len)
    lf = (wl - 1) // 2 + 1
    lb = wl // 2 + 1
    order = np.argsort(wl, kind="stable")

    Bc = B // NCORES
    ntiles = Bc // S
    lf_pc = lf[order].reshape(-1, NCORES)
    lb_pc = lb[order].reshape(-1, NCORES)

    def r64(v):
        return int(min(S, -(-int(v) // 64) * 64))

    mode = os.environ.get("GRU_SCHED", "exact")

    def dir_steps(lens_pc, t):
        seg = lens_pc[t * S:(t + 1) * S]  # [S, NCORES]
        n = int(seg.max())
        steps = []
        prev_cmax = 0
        prev_wr = 0
        for j in range(n):
            need = n - j
            cnt = (seg >= need).sum(axis=0)
            cmax, cmin = int(cnt.max()), int(cnt.min())
            if mode == "v1ish":
                wr = wz = int(min(S, max(256, r64(cmax))))
                wh = prev_wr if j > 0 else 0
            elif mode == "a":  # exact wr, wz==wr, wide wh
                wr = wz = r64(cmax)
                wh = prev_wr if j > 0 else 0
            elif mode == "b":  # clamped wr, narrow wh
                wr = wz = int(min(S, max(256, r64(cmax))))
                wh = r64(prev_cmax) if j > 0 else 0
            else:
                wr = r64(cmax)
                wz = wr if wr <= 128 else max(256, wr)
                if mode == "nowh":
                    wh = prev_wr if j > 0 else 0
                else:
                    wh = r64(prev_cmax) if j > 0 else 0
            wmask = wr - cmin  # width of the over-included (to-freeze) prefix
            steps.append((wr, wz, wh, wmask))
            prev_cmax = cmax
            prev_wr = wr
        return tuple(steps)

    sched = tuple((dir_steps(lf_pc, t), dir_steps(lb_pc, t))
                  for t in range(ntiles))

    cache_key = (sched, bool(os.environ.get("GRU_DBG")))
    if cache_key not in _PROGRAM_CACHE:
        _PROGRAM_CACHE[cache_key] = _build_program(sched)
    nc = _PROGRAM_CACHE[cache_key]

    f32 = np.float32
    WihfT, WhhfT = Wih_f.T.astype(f32), Whh_f.T.astype(f32)
    WihbT, WhhbT = Wih_b.T.astype(f32), Whh_b.T.astype(f32)
    wn_f = np.ascontiguousarray(WihfT[:, 2 * H:] * SW)
    wn_b = np.ascontiguousarray(WihbT[:, 2 * H:] * SW)
    w8i_f = (WihfT[:, :2 * H] * SW).astype(E4NP)
    w8i_b = (WihbT[:, :2 * H] * SW).astype(E4NP)
    w8h_f = (WhhfT * SW).astype(E4NP)
    w8h_b = (WhhbT * SW).astype(E4NP)
    w1 = np.ascontiguousarray(W1.T, dtype=f32)
    w2 = np.ascontiguousarray(W2.T, dtype=f32)

    def chunks(v):  # [512] -> [4, 128]
        return np.asarray(v, f32).reshape(4, 128)

    bias = np.concatenate([
        chunks((bih_f + bhh_f)[:H]), chunks(-(bih_f + bhh_f)[H:2 * H]),
        chunks(SW * bhh_f[2 * H:]), chunks(bih_f[2 * H:]),
        chunks((bih_b + bhh_b)[:H]), chunks(-(bih_b + bhh_b)[H:2 * H]),
        chunks(SW * bhh_b[2 * H:]), chunks(bih_b[2 * H:]),
        chunks(b1), chunks(b2),
    ], 0)  # [40, 128]

    pw = np.asarray(padded_window, f32)
    in_maps = []
    p8 = np.arange(8)
    for c in range(NCORES):
        idx = order[c::NCORES]
        xT = np.ascontiguousarray(pw[idx].transpose(1, 2, 0))  # [15, 512, Bc]
        mzf = (BIGM * (p8[:, None] < (8 - lf[idx])[None, :])).astype(f32)
        mzb = (BIGM * (p8[:, None] < (8 - lb[idx])[None, :])).astype(f32)
        in_maps.append({
            "xT": xT, "xT8": xT.astype(E4NP),
            "wnf": wn_f, "wnb": wn_b,
            "w8if": w8i_f, "w8ib": w8i_b, "w8hf": w8h_f, "w8hb": w8h_b,
            "w1": w1, "w2": w2,
            "bias": bias, "maskzf": mzf, "maskzb": mzb,
        })

    trace = bool(os.environ.get("GRU_TRACE"))
    kw = {}
    if os.environ.get("GRU_TMPDIR"):
        kw["tmpdir"] = os.environ["GRU_TMPDIR"]
    res = run_bass_kernel_spmd(nc, in_maps, core_ids=list(range(NCORES)),
                               trace=trace, **kw)
    global LAST_RESULT
    LAST_RESULT = res
    out = np.empty((B, H), f32)
    for c in range(NCORES):
        out[order[c::NCORES]] = res.results[c]["y"]
    return out
